# revision 31
# baseline (speedup 1.0000x reference)
"""Trainium2 Bass kernel for HeavilyCompressedAttention.

Sharding: 16 heads across 8 cores (2 heads/core, tensor-parallel);
compressed-KV path (single shared head) replicated on every core;
out_proj row-parallel with host-side partial sum (bf16 partials).

Design notes (cost-model driven):
  - Projections run weight-stationary producing TRANSPOSED outputs
    (qT/lkT/lvT = [d, s]) -- exactly the stationary layout the attention
    scores need, so there are no per-tile q/k transposes.
  - The compressed-entries matmuls, the raw squares, the w-fold and the
    RoPE rotation all overlap the projection phase (PSUM split 4+4
    banks; hN streamed behind hT in the in-order DMA queue).
  - RMSNorm is algebraic-folded: ssq reduced via PE selector matmuls +
    tiny transposes; 1/rms applied as a per-partition activation scale
    on the q side and a PE-outer-product broadcast fold on the k side.
  - Softmax: unmasked exp (logits are O(1) by construction), 0/1
    mask-multiplies on the GpSimd engine, free-dim reduces for the
    denominators, 1/den and the 0.5 branch weight folded into one
    two-scalar tensor_scalar.
  - Compressor softmax weights are host-prepared (0.016% of FLOPs).
  - Few, large DMAs; bf16 output partials summed on host.
"""

import os
import sys

import numpy as np
import ml_dtypes

for _p in ("/opt/trn_rl_repo", "/root/.axon_site/_ro/trn_rl_repo"):
    if os.path.isdir(_p) and _p not in sys.path:
        sys.path.insert(0, _p)

from concourse import bacc, mybir  # noqa: E402
import concourse.tile as tile  # noqa: E402
from concourse.bass_utils import run_bass_kernel_spmd  # noqa: E402
from concourse.masks import make_identity  # noqa: E402

F32 = mybir.dt.float32
BF16 = mybir.dt.bfloat16
NPBF = ml_dtypes.bfloat16
AF = mybir.ActivationFunctionType
ALU = mybir.AluOpType

S = 2048
HID = 2048
NH = 16
HD = 128
R = 16
C = S // R  # 128
WIN = 128
ROPE = HD // 2  # 64
HALF = ROPE // 2  # 32
EPS = 1e-6
NT = S // 128  # 16 s-tiles
KT = HID // 128  # 16 k-tiles
NCORES = 8
HPC = NH // NCORES  # 2 heads per core
SCALE = 1.0 / float(np.sqrt(HD))

_CACHE = {}


def _build_bass():
    nc = bacc.Bacc("TRN2", target_bir_lowering=False, debug=False,
                   num_devices=NCORES)

    din = {}

    def inp(name, shape, dt):
        din[name] = nc.dram_tensor(name, list(shape), dt, kind="ExternalInput")
        return din[name]

    hT = inp("hT", [KT, 128, S], BF16)        # hidden^T chunks [k][d, s]
    hN = inp("hN", [NT, 128, HID], BF16)      # hidden natural s-tiles
    w6 = inp("w6", [6, 128, 2048], BF16)      # [q0|q1|lk0|lk1|lv0|lv1] chunks
    b6w = inp("b6w", [128, 8], F32)           # biases (6 cols) | qn_w | kn_w
    tabs = inp("tabs", [64, S], BF16)         # rope tables [cos;sin][freq, pos]
    wbig = inp("wbig", [128, NT * 128], BF16)  # host-softmaxed compressor wts
    wkv = inp("wkv", [128, KT * 256], BF16)   # [Wk|Wv] shared head
    bkv = inp("bkv", [1, 256], BF16)
    wo = inp("wo", [128, HPC * HID], BF16)    # Wo rows per head [d][h, o]
    ckro = inp("ckro", [C, 192], F32)         # ck rope tabs A|B|C|D|pass
    ish = inp("ish", [128, 256], BF16)        # identity | shift(+1)
    loc01 = inp("loc01", [128, 256], BF16)    # local window 0/1 mask
    cm01 = inp("cm01", [128, NT * 129], BF16)  # compressed 0/1 mask (+sink col)
    skT = inp("skT", [128, HPC], BF16)        # sink_k columns
    sv = inp("sv", [HPC, 128], BF16)          # sink_v rows

    out_p = nc.dram_tensor("out_p", [NT, 128, HID], BF16, kind="ExternalOutput")

    with tile.TileContext(nc) as tc:
        with (
            tc.tile_pool(name="const", bufs=1) as cst,
            tc.tile_pool(name="persist", bufs=1) as per,
            tc.tile_pool(name="stats", bufs=4) as sts,
        ):
            # ---------------- consts ----------------
            def load(pool, name, shape, dt):
                t = pool.tile(list(shape), dt, name=f"c_{name}")
                nc.sync.dma_start(out=t[:], in_=din[name].ap())
                return t

            ident_f32 = cst.tile([128, 128], F32)
            make_identity(nc, ident_f32[:])
            onesrow = cst.tile([1, 128], BF16)
            nc.vector.memset(onesrow[:], 1.0)
            eps_t = cst.tile([128, 1], F32)
            nc.vector.memset(eps_t[:], EPS)
            sel_sb = cst.tile([128, 16], BF16)
            nc.vector.memset(sel_sb[:], 0.0)
            for t4 in range(4):
                nc.vector.memset(sel_sb[:, 5 * t4:5 * t4 + 1], 1.0)

            # ---------------- persistent activations ----------------
            qlkT = per.tile([128, 4, S], BF16)     # roped q|lk transposed
            lvn = per.tile([128, NT, 256], BF16)   # lv natural
            rsc = per.tile([128, 64], F32)         # 1/rms  [(i,tensor)]
            rflat = per.tile([1, 32 * 128], BF16)  # k-col 1/rms on partition 0
            entries = per.tile([C, HID], BF16)
            eT = per.tile([128, KT, C], BF16)
            cvn = per.tile([C, 128], BF16)
            sinkcv = per.tile([128, HPC, 128], BF16)
            ckT = per.tile([128, C], BF16)
            cv127 = per.tile([1, 128], BF16)

            # ====== P1 (+ entries, squares, rope, all overlapped) =======
            with tc.tile_pool(name="mid", bufs=1) as mid:
                lvT = mid.tile([128, HPC, S], BF16)   # lv transposed
                sqall = mid.tile([128, 4, S], BF16)   # raw squares for ssq
                rscT = mid.tile([32, 128], BF16)      # k-cols of 1/rms, transp
                with (
                    tc.tile_pool(name="scrA", bufs=2) as scrA,
                    tc.tile_pool(name="bulk", bufs=1) as blk,
                    tc.tile_pool(name="ps_p1", bufs=1, space="PSUM") as pq,
                    tc.tile_pool(name="ps_e", bufs=1, space="PSUM") as pe,
                ):
                    w6_sb = blk.tile([128, 6, 2048], BF16)
                    hT_sb = blk.tile([128, KT, S], BF16)
                    # DMA issue order matters (SP queue is in-order):
                    # w6[0], wbig, hT chunks, w6 rest, early consts, hN ring
                    nc.sync.dma_start(out=w6_sb[:, 0, :], in_=w6.ap()[0])
                    for k in range(KT):
                        nc.sync.dma_start(out=hT_sb[:, k, :], in_=hT.ap()[k])
                    b6w_sb = load(cst, "b6w", [128, 8], F32)
                    wbig_sb = load(cst, "wbig", [128, NT, 128], BF16)
                    for f in range(1, 6):
                        nc.sync.dma_start(out=w6_sb[:, f, :], in_=w6.ap()[f])
                    tabs_sb = load(cst, "tabs", [64, S], BF16)
                    sinT0 = cst.tile([HALF, S], BF16)
                    nc.sync.dma_start(out=sinT0[:],
                                      in_=din["tabs"].ap()[HALF:ROPE, :])
                    hns = []
                    for i in range(NT):
                        hn = scrA.tile([128, HID], BF16, tag="hN", bufs=4)
                        nc.sync.dma_start(out=hn[:], in_=hN.ap()[i])
                        hns.append(hn)
                    ish_sb = load(cst, "ish", [128, 256], BF16)

                    # --- projections ---
                    for f in range(6):
                        ps = pq.tile([128, 2048], F32, name="p1")
                        for k in range(KT):
                            for sq in range(4):
                                nc.tensor.matmul(
                                    ps[:, sq * 512:(sq + 1) * 512],
                                    w6_sb[:, f, k * 128:(k + 1) * 128],
                                    hT_sb[:, k, sq * 512:(sq + 1) * 512],
                                    start=(k == 0), stop=(k == KT - 1))
                        if f < 4:
                            dst = qlkT[:, f, :]
                        else:
                            dst = lvT[:, f - 4, :]
                        bias = b6w_sb[:, f:f + 1]
                        nc.scalar.activation(dst[0:128, 0:1024], ps[:, 0:1024],
                                             AF.Identity, bias=bias)
                        nc.vector.tensor_scalar_add(dst[0:128, 1024:2048],
                                                    ps[:, 1024:2048], bias)
                        if f < 4:
                            nc.vector.tensor_mul(sqall[:, f, :], dst, dst)

                    # --- compressed entries (streams hN, interleaves) ---
                    ps_e = pe.tile([C, HID], F32)
                    for i in range(NT):
                        for hc in range(4):
                            nc.tensor.matmul(ps_e[:, hc * 512:(hc + 1) * 512],
                                             wbig_sb[:, i, :],
                                             hns[i][:, hc * 512:(hc + 1) * 512],
                                             start=(i == 0), stop=(i == NT - 1))
                    nc.scalar.copy(entries[:, 0:1024], ps_e[:, 0:1024])
                    nc.vector.tensor_copy(entries[:, 1024:2048],
                                          ps_e[:, 1024:2048])

                    # --- w-fold + rope (overlaps P1 on DVE; x2 half is
                    # shuttled to partitions 0:32 via SBUF->SBUF DMA) ---
                    cosT = tabs_sb[0:HALF, :]
                    for t4 in range(4):
                        wcol = b6w_sb[:, 6:7] if t4 < 2 else b6w_sb[:, 7:8]
                        nc.vector.tensor_scalar_mul(qlkT[:, t4, :],
                                                    qlkT[:, t4, :], wcol)
                        x1 = qlkT[0:HALF, t4, :]
                        x2d = scrA.tile([HALF, S], BF16, tag="x2d", bufs=1)
                        nc.sync.dma_start(out=x2d[:],
                                          in_=qlkT[HALF:ROPE, t4, :])
                        ta = scrA.tile([HALF, S], BF16, tag="ta", bufs=1)
                        tb = scrA.tile([HALF, S], BF16, tag="tb", bufs=1)
                        tc2 = scrA.tile([HALF, S], BF16, tag="tc2", bufs=1)
                        nc.vector.tensor_mul(ta[:], x1, cosT)
                        nc.vector.tensor_mul(tc2[:], x1, sinT0[:])
                        nc.vector.tensor_mul(tb[:], x2d[:], sinT0[:])
                        nc.vector.tensor_sub(x1, ta[:], tb[:])
                        nc.vector.tensor_mul(ta[:], x2d[:], cosT)
                        nc.vector.tensor_add(tb[:], tc2[:], ta[:])
                        nc.sync.dma_start(out=qlkT[HALF:ROPE, t4, :],
                                          in_=tb[:])

                # ---- mini: norms, k-fold, lv transpose, entries/kv ----
                with (
                    tc.tile_pool(name="scrM", bufs=2) as scrM,
                    tc.tile_pool(name="ps_ssq", bufs=1, space="PSUM") as pssq,
                    tc.tile_pool(name="ps_rsc", bufs=1, space="PSUM") as prsc,
                    tc.tile_pool(name="ps_tp", bufs=2, space="PSUM") as ptp,
                    tc.tile_pool(name="ps_bb", bufs=1, space="PSUM") as pbb,
                    tc.tile_pool(name="ps_kv", bufs=1, space="PSUM") as pkvp,
                ):
                    wkv_sb = load(scrM, "wkv", [128, KT, 256], BF16)
                    bkv_sb = load(scrM, "bkv", [1, 256], BF16)
                    ckro_sb = load(scrM, "ckro", [C, 192], F32)
                    sv_sb = load(scrM, "sv", [1, HPC * 128], BF16)
                    rsc_ps = prsc.tile([128, 64], F32)
                    for qtr in range(4):
                        ssq_ps = pssq.tile([4, 512], F32, name="ssq")
                        for t4 in range(4):
                            nc.tensor.matmul(
                                ssq_ps[:], sel_sb[:, 4 * t4:4 * t4 + 4],
                                sqall[:, t4, qtr * 512:(qtr + 1) * 512],
                                start=(t4 == 0), stop=(t4 == 3))
                        ssq_sb = scrM.tile([4, 512], F32, tag="ssqs")
                        nc.scalar.copy(ssq_sb[:], ssq_ps[:])
                        for j in range(4):
                            i = qtr * 4 + j
                            nc.tensor.matmul(
                                rsc_ps[:, 4 * i:4 * i + 4],
                                ssq_sb[0:4, j * 128:(j + 1) * 128],
                                ident_f32[0:4, 0:4], is_transpose=True,
                                start=(i == 0), stop=(i == NT - 1))
                    rms_sb = sts.tile([128, 64], F32)
                    nc.scalar.activation(rms_sb[:], rsc_ps[:], AF.Sqrt,
                                         scale=1.0 / HD, bias=eps_t[:])
                    nc.vector.reciprocal(rsc[:], rms_sb[:])
                    # fold softmax scale into the q columns only
                    rsc4 = rsc[:].rearrange("p (i t) -> p i t", t=4)
                    nc.vector.tensor_scalar_mul(rsc4[:, :, 0:2],
                                                rsc4[:, :, 0:2], SCALE)
                    # transpose the k columns, then flatten onto partition 0
                    rsck = sts.tile([128, 32], F32)
                    nc.vector.tensor_copy(
                        rsck[:].rearrange("p (i t) -> p i t", t=2),
                        rsc4[:, :, 2:4])
                    rT_ps = ptp.tile([128, 128], F32, tag="tp")
                    nc.tensor.matmul(rT_ps[0:32, :], rsck[:], ident_f32[:],
                                     is_transpose=True, start=True, stop=True)
                    nc.scalar.copy(rscT[:], rT_ps[0:32, :])
                    nc.sync.dma_start(out=rflat[:], in_=rscT[:])

                    # k-side 1/rms broadcast fold: lkT *= bcast(rsc_k)
                    for h in range(HPC):
                        bb_sb = scrM.tile([128, S], BF16, tag="bb", bufs=1)
                        for qtr in range(4):
                            bb_ps = pbb.tile([128, 512], F32, name="bb")
                            for j in range(4):
                                i = qtr * 4 + j
                                r = i * 2 + h
                                nc.tensor.matmul(
                                    bb_ps[:, j * 128:(j + 1) * 128],
                                    onesrow[:],
                                    rflat[:, r * 128:(r + 1) * 128],
                                    start=(j == 0), stop=(j == 3))
                            if qtr % 2 == 0:
                                nc.scalar.copy(
                                    bb_sb[:, qtr * 512:(qtr + 1) * 512],
                                    bb_ps[:])
                            else:
                                nc.vector.tensor_copy(
                                    bb_sb[:, qtr * 512:(qtr + 1) * 512],
                                    bb_ps[:])
                        nc.vector.tensor_mul(qlkT[:, 2 + h, :],
                                             qlkT[:, 2 + h, :], bb_sb[:])

                    # lv natural via PE transposes
                    for h in range(HPC):
                        for ti in range(NT):
                            tp = ptp.tile([128, 128], BF16, tag="tpl")
                            nc.tensor.matmul(
                                tp[:], lvT[:, h, ti * 128:(ti + 1) * 128],
                                ish_sb[:, 0:128], is_transpose=True,
                                start=True, stop=True)
                            if ti % 2 == 0:
                                nc.vector.tensor_copy(
                                    lvn[:, ti, h * 128:(h + 1) * 128], tp[:])
                            else:
                                nc.scalar.copy(
                                    lvn[:, ti, h * 128:(h + 1) * 128], tp[:])

                    # entries^T + shared ck/cv head
                    for k in range(KT):
                        tp = ptp.tile([128, 128], BF16, tag="tp")
                        nc.tensor.matmul(tp[:],
                                         entries[:, k * 128:(k + 1) * 128],
                                         ish_sb[:, 0:128], is_transpose=True,
                                         start=True, stop=True)
                        if k % 2 == 0:
                            nc.vector.tensor_copy(eT[:, k, :], tp[:])
                        else:
                            nc.scalar.copy(eT[:, k, :], tp[:])

                    ps_kv = pkvp.tile([C, 256], F32)
                    for k in range(KT):
                        nc.tensor.matmul(ps_kv[:], eT[:, k, :], wkv_sb[:, k, :],
                                         start=(k == 0), stop=False)
                    nc.tensor.matmul(ps_kv[:], onesrow[:], bkv_sb[:],
                                     start=False, stop=True)

                    # ck: rmsnorm + rope at block-end positions
                    sqc = scrM.tile([C, 128], F32, tag="sqc")
                    ssqc = sts.tile([C, 1], F32)
                    nc.scalar.activation(sqc[:], ps_kv[:, 0:128], AF.Square,
                                         accum_out=ssqc[:])
                    rmsc = sts.tile([C, 1], F32)
                    nc.scalar.activation(rmsc[:], ssqc[:], AF.Sqrt,
                                         scale=1.0 / HD, bias=eps_t[:])
                    rscc = sts.tile([C, 1], F32)
                    nc.vector.reciprocal(rscc[:], rmsc[:])
                    ckR = scrM.tile([C, 128], BF16, tag="ckR")
                    c1 = scrM.tile([C, HALF], F32, tag="ckt1")
                    c2 = scrM.tile([C, HALF], F32, tag="ckt2")
                    nc.vector.tensor_mul(c1[:], ps_kv[:, 0:HALF],
                                         ckro_sb[:, 0:32])
                    nc.vector.tensor_mul(c2[:], ps_kv[:, HALF:ROPE],
                                         ckro_sb[:, 32:64])
                    nc.vector.tensor_sub(ckR[:, 0:HALF], c1[:], c2[:])
                    nc.vector.tensor_mul(c1[:], ps_kv[:, 0:HALF],
                                         ckro_sb[:, 64:96])
                    nc.vector.tensor_mul(c2[:], ps_kv[:, HALF:ROPE],
                                         ckro_sb[:, 96:128])
                    nc.vector.tensor_add(ckR[:, HALF:ROPE], c1[:], c2[:])
                    nc.vector.tensor_mul(ckR[:, ROPE:128], ps_kv[:, ROPE:128],
                                         ckro_sb[:, 128:192])
                    nc.vector.tensor_scalar_mul(ckR[:], ckR[:], rscc[:])
                    tpc = ptp.tile([128, 128], BF16, tag="tp")
                    nc.tensor.matmul(tpc[:], ckR[:], ish_sb[:, 0:128],
                                     is_transpose=True, start=True, stop=True)
                    nc.vector.tensor_copy(ckT[:], tpc[:])

                    nc.scalar.copy(cvn[:], ps_kv[:, 128:256])
                    nc.sync.dma_start(out=cv127[:], in_=cvn[127:128, :])
                    for h in range(HPC):
                        tps = ptp.tile([128, 128], F32, tag="tp")
                        nc.tensor.matmul(tps[:], ish_sb[:, 128:256], cvn[:],
                                         start=True, stop=False)
                        nc.tensor.matmul(tps[:], ish_sb[0:1, 0:128],
                                         sv_sb[0:1, h * 128:(h + 1) * 128],
                                         start=False, stop=True)
                        nc.scalar.copy(sinkcv[:, h, :], tps[:])

            # ============ late phases (P3 + P4) ==========================
            with (
                tc.tile_pool(name="late", bufs=1) as late,
                tc.tile_pool(name="scrB", bufs=2) as scrB,
                tc.tile_pool(name="ps_sc", bufs=2, space="PSUM") as psc,
                tc.tile_pool(name="ps_pt", bufs=2, space="PSUM") as pptp,
                tc.tile_pool(name="ps_cx", bufs=2, space="PSUM") as pcx,
                tc.tile_pool(name="ps_o", bufs=2, space="PSUM") as pout,
            ):
                mgT = late.tile([128, HPC, S], BF16)   # merged^T
                loc01_sb = load(late, "loc01", [128, 256], BF16)
                cm_sb = load(late, "cm01", [128, NT, 129], BF16)
                skT_sb = load(late, "skT", [128, HPC], BF16)
                wo_sb = load(late, "wo", [128, HPC, HID], BF16)

                for i in range(NT):
                    lo = 0 if i > 0 else 128
                    den = sts.tile([128, 4], F32)
                    rden = sts.tile([128, 4], F32)
                    p_sb = scrB.tile([128, HPC, 385], BF16, tag="p")
                    for h in range(HPC):
                        ps_s = psc.tile([128, 385], F32, name="sc")
                        qT = qlkT[:, h, i * 128:(i + 1) * 128]
                        if i > 0:
                            nc.tensor.matmul(
                                ps_s[:, 0:256], qT,
                                qlkT[:, 2 + h, (i - 1) * 128:(i + 1) * 128],
                                start=True, stop=True)
                        else:
                            nc.tensor.matmul(ps_s[:, 128:256], qT,
                                             qlkT[:, 2 + h, 0:128],
                                             start=True, stop=True)
                        nc.tensor.matmul(ps_s[:, 256:257], qT,
                                         skT_sb[:, h:h + 1],
                                         start=True, stop=True)
                        nc.tensor.matmul(ps_s[:, 257:385], qT, ckT[:],
                                         start=True, stop=True)
                        nc.scalar.activation(
                            p_sb[:, h, lo:385], ps_s[:, lo:385], AF.Exp,
                            scale=rsc[:, 4 * i + h:4 * i + h + 1])
                    locb = (loc01_sb[:, lo:256].unsqueeze(1)
                            .broadcast_to([128, HPC, 256 - lo]))
                    nc.gpsimd.tensor_mul(p_sb[:, :, lo:256],
                                         p_sb[:, :, lo:256], locb)
                    cmb = (cm_sb[:, i, :].unsqueeze(1)
                           .broadcast_to([128, HPC, 129]))
                    nc.gpsimd.tensor_mul(p_sb[:, :, 256:385],
                                         p_sb[:, :, 256:385], cmb)
                    # den cols: [local h0, local h1, comp h0, comp h1]
                    nc.vector.tensor_reduce(den[:, 0:2], p_sb[:, :, lo:256],
                                            mybir.AxisListType.X, ALU.add)
                    nc.vector.tensor_reduce(den[:, 2:4], p_sb[:, :, 256:385],
                                            mybir.AxisListType.X, ALU.add)
                    nc.vector.reciprocal(rden[:], den[:])
                    for h in range(HPC):
                        ctx = pcx.tile([128, 128], F32, name="cx")
                        lo2 = 0 if i > 0 else 128
                        nc.gpsimd.tensor_scalar(
                            out=p_sb[:, h, lo2:256],
                            in0=p_sb[:, h, lo2:256],
                            scalar1=rden[:, h:h + 1], scalar2=0.5,
                            op0=ALU.mult, op1=ALU.mult)
                        nc.vector.tensor_scalar(
                            out=p_sb[:, h, 256:385],
                            in0=p_sb[:, h, 256:385],
                            scalar1=rden[:, 2 + h:3 + h],
                            scalar2=0.5, op0=ALU.mult, op1=ALU.mult)
                        pt = pptp.tile([128, 4, 128], BF16, name="pt")
                        last15 = (i == NT - 1)
                        if i > 0:
                            nc.tensor.matmul(pt[:, 0, :], p_sb[:, h, 0:128],
                                             ish_sb[:, 0:128],
                                             is_transpose=True,
                                             start=True, stop=False)
                        nc.tensor.matmul(pt[:, 1, :], p_sb[:, h, 128:256],
                                         ish_sb[:, 0:128],
                                         is_transpose=True,
                                         start=(i == 0), stop=False)
                        if last15:
                            nc.tensor.matmul(pt[0:1, 3, :],
                                             p_sb[:, h, 384:385],
                                             ish_sb[:, 0:128],
                                             is_transpose=True,
                                             start=False, stop=False)
                        nc.tensor.matmul(pt[:, 2, :], p_sb[:, h, 256:384],
                                         ish_sb[:, 0:128],
                                         is_transpose=True,
                                         start=False, stop=True)
                        ptsb = scrB.tile([128, 4, 128], BF16, tag="pt")
                        c0 = 0 if i > 0 else 1
                        if h == 0:
                            nc.scalar.copy(ptsb[:, c0:3, :], pt[:, c0:3, :])
                        else:
                            nc.vector.tensor_copy(ptsb[:, c0:3, :],
                                                  pt[:, c0:3, :])
                        if last15:
                            nc.vector.tensor_copy(ptsb[0:1, 3, :],
                                                  pt[0:1, 3, :])
                        if i > 0:
                            nc.tensor.matmul(
                                ctx[:], lvn[:, i - 1, h * 128:(h + 1) * 128],
                                ptsb[:, 0, :], start=True, stop=False)
                        nc.tensor.matmul(ctx[:],
                                         lvn[:, i, h * 128:(h + 1) * 128],
                                         ptsb[:, 1, :], start=(i == 0),
                                         stop=False)
                        nc.tensor.matmul(ctx[:], sinkcv[:, h, :],
                                         ptsb[:, 2, :], start=False,
                                         stop=not last15)
                        if last15:
                            nc.tensor.matmul(ctx[:], cv127[:],
                                             ptsb[0:1, 3, :], start=False,
                                             stop=True)
                        if h == 0:
                            nc.scalar.copy(mgT[:, h, i * 128:(i + 1) * 128],
                                           ctx[:])
                        else:
                            nc.vector.tensor_copy(
                                mgT[:, h, i * 128:(i + 1) * 128], ctx[:])

                    # ---- P4 for this s-tile ----
                    o_sb = scrB.tile([128, HID], BF16, tag="o")
                    for qo in range(4):
                        po = pout.tile([128, 512], F32, name="o4")
                        for h in range(HPC):
                            nc.tensor.matmul(
                                po[:], mgT[:, h, i * 128:(i + 1) * 128],
                                wo_sb[:, h, qo * 512:(qo + 1) * 512],
                                start=(h == 0), stop=(h == HPC - 1))
                        # 1024 cols on ACT, 1024 on DVE (engine balance)
                        off = qo * 512
                        if qo < 2:
                            nc.scalar.copy(o_sb[:, off:off + 512], po[:])
                        else:
                            nc.vector.tensor_copy(o_sb[:, off:off + 512],
                                                  po[:])
                    nc.sync.dma_start(out=out_p.ap()[i], in_=o_sb[:])

    nc.compile()
    return nc


def _host_prep(inputs):
    """Build the 8 per-core input maps from full inputs."""
    hs = np.asarray(inputs["hidden_states"], np.float32)[0]  # [S, HID]
    Wq = np.asarray(inputs["Wq"], np.float32)
    Wc = np.asarray(inputs["Wc"], np.float32)
    Wk = np.asarray(inputs["Wk"], np.float32)
    Wv = np.asarray(inputs["Wv"], np.float32)
    Wlk = np.asarray(inputs["Wlk"], np.float32)
    Wlv = np.asarray(inputs["Wlv"], np.float32)
    qn_w = np.asarray(inputs["qn_w"], np.float32)
    kn_w = np.asarray(inputs["kn_w"], np.float32)
    sink_k = np.asarray(inputs["sink_k"], np.float32)
    sink_v = np.asarray(inputs["sink_v"], np.float32)
    Wo = np.asarray(inputs["Wo"], np.float32)
    bq = np.asarray(inputs["bq"], np.float32)
    bc = np.asarray(inputs["bc"], np.float32)
    bk = np.asarray(inputs["bk"], np.float32)
    bv = np.asarray(inputs["bv"], np.float32)
    blk = np.asarray(inputs["blk"], np.float32)
    blv = np.asarray(inputs["blv"], np.float32)

    hT_t = np.ascontiguousarray(hs.T.reshape(KT, 128, S)).astype(NPBF)
    hN_t = hs.reshape(NT, 128, HID).astype(NPBF)

    # host-side compressor softmax (0.016% of FLOPs)
    cw = hs @ Wc[:, 0] + bc[0]                      # [S]
    cwb = cw.reshape(C, R)
    e = np.exp(cwb - cwb.max(axis=1, keepdims=True))
    w = e / e.sum(axis=1, keepdims=True)            # [C, R]
    s_idx = np.arange(S)
    wbig = np.zeros((128, NT, 128), np.float32)
    wbig[s_idx % 128, s_idx // 128, s_idx // R] = w[s_idx // R, s_idx % R]
    wbig = wbig.reshape(128, NT * 128).astype(NPBF)

    # rope tables [freq, pos]
    pos = np.arange(S, dtype=np.float32)
    inv_freq = 1.0 / (10000.0 ** (np.arange(HALF, dtype=np.float32) * 2.0 / ROPE))
    ang = inv_freq[:, None] * pos[None, :]          # [HALF, S]
    tabs = np.concatenate([np.cos(ang), np.sin(ang)], axis=0).astype(NPBF)

    # ck rope tables at block-end positions (kn_w folded)
    pos_c = (np.arange(C, dtype=np.float32) * R + (R - 1))
    angc = pos_c[:, None] * inv_freq[None, :]       # [C, HALF]
    cosc, sinc = np.cos(angc), np.sin(angc)
    kw1, kw2 = kn_w[0:HALF], kn_w[HALF:ROPE]
    ckro = np.concatenate([
        cosc * kw1[None, :], sinc * kw2[None, :],
        sinc * kw1[None, :], cosc * kw2[None, :],
        np.broadcast_to(kn_w[ROPE:][None, :], (C, ROPE)),
    ], axis=1).astype(np.float32)

    # identity | shift(+1)
    r = np.arange(128)
    ident = (r[:, None] == r[None, :]).astype(np.float32)
    shift = (r[None, :] == r[:, None] + 1).astype(np.float32)
    ish = np.concatenate([ident, shift], axis=1).astype(NPBF)

    # masks (multiplicative 0/1)
    p = r[:, None]
    j = np.arange(256)[None, :]
    loc01 = np.where(j < 128, (j >= p), (j - 128 <= p)).astype(np.float32)
    loc01 = loc01.astype(NPBF)
    cm = np.zeros((128, NT, 129), np.float32)
    cm[:, :, 0] = 1.0
    ii = np.arange(NT)[None, :, None]
    cc = np.arange(128)[None, None, :]
    cm[:, :, 1:] = (cc * R + (R - 1) <= ii * 128 + p[:, None]).astype(np.float32)
    cm = cm.reshape(128, NT * 129).astype(NPBF)

    wkv_h = np.ascontiguousarray(
        np.concatenate([Wk, Wv], axis=1).reshape(KT, 128, 256)
        .transpose(1, 0, 2)).reshape(128, KT * 256).astype(NPBF)

    common = dict(hT=hT_t, hN=hN_t, tabs=tabs, wbig=wbig, wkv=wkv_h,
                  bkv=np.concatenate([bk, bv])[None, :].astype(NPBF),
                  ckro=ckro, ish=ish, loc01=loc01, cm01=cm)

    Wq4 = Wq.reshape(HID, NH, HD)
    Wlk4 = Wlk.reshape(HID, NH, HD)
    Wlv4 = Wlv.reshape(HID, NH, HD)
    bq4 = bq.reshape(NH, HD)
    blk4 = blk.reshape(NH, HD)
    blv4 = blv.reshape(NH, HD)
    Wo4 = Wo.reshape(NH, HD, HID)

    def tchunk(wcol):  # [HID, 128] -> [128, 2048] transposed-chunk layout
        return np.ascontiguousarray(
            wcol.reshape(KT, 128, 128).transpose(1, 0, 2)).reshape(128, 2048)

    in_maps = []
    for c in range(NCORES):
        hh = [HPC * c + h for h in range(HPC)]
        w6 = np.stack([
            tchunk(Wq4[:, hh[0]]), tchunk(Wq4[:, hh[1]]),
            tchunk(Wlk4[:, hh[0]]), tchunk(Wlk4[:, hh[1]]),
            tchunk(Wlv4[:, hh[0]]), tchunk(Wlv4[:, hh[1]]),
        ]).astype(NPBF)
        b6w = np.stack([bq4[hh[0]], bq4[hh[1]], blk4[hh[0]], blk4[hh[1]],
                        blv4[hh[0]], blv4[hh[1]], qn_w, kn_w],
                       axis=1).astype(np.float32)
        wo_c = np.ascontiguousarray(
            Wo4[hh].transpose(1, 0, 2)).reshape(128, HPC * HID).astype(NPBF)
        m = dict(common)
        m.update(w6=w6, b6w=b6w, wo=wo_c,
                 skT=sink_k[hh].T.astype(NPBF),
                 sv=sink_v[hh].astype(NPBF))
        in_maps.append(m)
    return in_maps


def kernel(**inputs):
    if "nc" not in _CACHE:
        _CACHE["nc"] = _build_bass()
    nc = _CACHE["nc"]
    in_maps = _host_prep(inputs)
    res = run_bass_kernel_spmd(nc, in_maps, core_ids=list(range(NCORES)))
    out = np.zeros((S, HID), np.float64)
    for c in range(NCORES):
        out += res.results[c]["out_p"].reshape(S, HID).astype(np.float64)
    out += np.asarray(inputs["bo"], np.float32)[None, :]
    return out[None].astype(np.float32)


# revision 32
# speedup vs baseline: 1.0160x; 1.0160x over previous
"""Trainium2 Bass kernel for HeavilyCompressedAttention.

Sharding: 16 heads across 8 cores (2 heads/core, tensor-parallel);
compressed-KV path (single shared head) replicated on every core;
out_proj row-parallel with host-side partial sum (bf16 partials).

Design notes (cost-model driven):
  - Projections run weight-stationary producing TRANSPOSED outputs
    (qT/lkT/lvT = [d, s]) -- exactly the stationary layout the attention
    scores need, so there are no per-tile q/k transposes.
  - The compressed-entries matmuls, the raw squares, the w-fold and the
    RoPE rotation all overlap the projection phase (PSUM split 4+4
    banks; hN streamed behind hT in the in-order DMA queue).
  - RMSNorm is algebraic-folded: ssq reduced via PE selector matmuls +
    tiny transposes; 1/rms applied as a per-partition activation scale
    on the q side and a PE-outer-product broadcast fold on the k side.
  - Softmax: unmasked exp (logits are O(1) by construction), 0/1
    mask-multiplies on the GpSimd engine, free-dim reduces for the
    denominators, 1/den and the 0.5 branch weight folded into one
    two-scalar tensor_scalar.
  - Compressor softmax weights are host-prepared (0.016% of FLOPs).
  - Few, large DMAs; bf16 output partials summed on host.
"""

import os
import sys

import numpy as np
import ml_dtypes

for _p in ("/opt/trn_rl_repo", "/root/.axon_site/_ro/trn_rl_repo"):
    if os.path.isdir(_p) and _p not in sys.path:
        sys.path.insert(0, _p)

from concourse import bacc, mybir  # noqa: E402
import concourse.tile as tile  # noqa: E402
from concourse.bass_utils import run_bass_kernel_spmd  # noqa: E402
from concourse.masks import make_identity  # noqa: E402

F32 = mybir.dt.float32
BF16 = mybir.dt.bfloat16
NPBF = ml_dtypes.bfloat16
AF = mybir.ActivationFunctionType
ALU = mybir.AluOpType

S = 2048
HID = 2048
NH = 16
HD = 128
R = 16
C = S // R  # 128
WIN = 128
ROPE = HD // 2  # 64
HALF = ROPE // 2  # 32
EPS = 1e-6
NT = S // 128  # 16 s-tiles
KT = HID // 128  # 16 k-tiles
NCORES = 8
HPC = NH // NCORES  # 2 heads per core
SCALE = 1.0 / float(np.sqrt(HD))

_CACHE = {}


def _build_bass():
    nc = bacc.Bacc("TRN2", target_bir_lowering=False, debug=False,
                   num_devices=NCORES)

    din = {}

    def inp(name, shape, dt):
        din[name] = nc.dram_tensor(name, list(shape), dt, kind="ExternalInput")
        return din[name]

    hT = inp("hT", [KT, 128, S], BF16)        # hidden^T chunks [k][d, s]
    hN = inp("hN", [NT, 128, HID], BF16)      # hidden natural s-tiles
    w6 = inp("w6", [6, 128, 2048], BF16)      # [q0|q1|lk0|lk1|lv0|lv1] chunks
    b6w = inp("b6w", [128, 8], F32)           # biases (6 cols) | qn_w | kn_w
    tabs = inp("tabs", [64, S], BF16)         # rope tables [cos;sin][freq, pos]
    wbig = inp("wbig", [128, NT * 128], BF16)  # host-softmaxed compressor wts
    wkv = inp("wkv", [128, KT * 256], BF16)   # [Wk|Wv] shared head
    bkv = inp("bkv", [1, 256], BF16)
    wo = inp("wo", [128, HPC * HID], BF16)    # Wo rows per head [d][h, o]
    ckro = inp("ckro", [C, 192], F32)         # ck rope tabs A|B|C|D|pass
    ish = inp("ish", [128, 256], BF16)        # identity | shift(+1)
    loc01 = inp("loc01", [128, 256], BF16)    # local window 0/1 mask
    cm01 = inp("cm01", [128, NT * 129], BF16)  # compressed 0/1 mask (+sink col)
    skT = inp("skT", [128, HPC], BF16)        # sink_k columns
    sv = inp("sv", [HPC, 128], BF16)          # sink_v rows

    out_p = nc.dram_tensor("out_p", [NT, 128, HID], BF16, kind="ExternalOutput")

    with tile.TileContext(nc) as tc:
        with (
            tc.tile_pool(name="const", bufs=1) as cst,
            tc.tile_pool(name="persist", bufs=1) as per,
            tc.tile_pool(name="stats", bufs=4) as sts,
        ):
            # ---------------- consts ----------------
            def load(pool, name, shape, dt):
                t = pool.tile(list(shape), dt, name=f"c_{name}")
                nc.sync.dma_start(out=t[:], in_=din[name].ap())
                return t

            ident_f32 = cst.tile([128, 128], F32)
            make_identity(nc, ident_f32[:])
            onesrow = cst.tile([1, 128], BF16)
            nc.vector.memset(onesrow[:], 1.0)
            eps_t = cst.tile([128, 1], F32)
            nc.vector.memset(eps_t[:], EPS)
            sel_sb = cst.tile([128, 16], BF16)
            nc.vector.memset(sel_sb[:], 0.0)
            for t4 in range(4):
                nc.vector.memset(sel_sb[:, 5 * t4:5 * t4 + 1], 1.0)

            # ---------------- persistent activations ----------------
            qlkT = per.tile([128, 4, S], BF16)     # roped q|lk transposed
            lvn = per.tile([128, NT, 256], BF16)   # lv natural
            rsc = per.tile([128, 64], F32)         # 1/rms  [(i,tensor)]
            rflat = per.tile([1, 32 * 128], BF16)  # k-col 1/rms on partition 0
            entries = per.tile([C, HID], BF16)
            eT = per.tile([128, KT, C], BF16)
            cvn = per.tile([C, 128], BF16)
            sinkcv = per.tile([128, HPC, 128], BF16)
            ckT = per.tile([128, C], BF16)
            cv127 = per.tile([1, 128], BF16)

            # ====== P1 (+ entries, squares, rope, all overlapped) =======
            with tc.tile_pool(name="mid", bufs=1) as mid:
                lvT = mid.tile([128, HPC, S], BF16)   # lv transposed
                sqall = mid.tile([128, 4, S], BF16)   # raw squares for ssq
                rscT = mid.tile([32, 128], BF16)      # k-cols of 1/rms, transp
                with (
                    tc.tile_pool(name="scrA", bufs=2) as scrA,
                    tc.tile_pool(name="bulk", bufs=1) as blk,
                    tc.tile_pool(name="ps_p1", bufs=1, space="PSUM") as pq,
                    tc.tile_pool(name="ps_e", bufs=1, space="PSUM") as pe,
                ):
                    w6_sb = blk.tile([128, 6, 2048], BF16)
                    hT_sb = blk.tile([128, KT, S], BF16)
                    # DMA issue order matters (SP queue is in-order):
                    # w6[0], wbig, hT chunks, w6 rest, early consts, hN ring
                    nc.sync.dma_start(out=w6_sb[:, 0, :], in_=w6.ap()[0])
                    for k in range(KT):
                        nc.sync.dma_start(out=hT_sb[:, k, :], in_=hT.ap()[k])
                    b6w_sb = load(cst, "b6w", [128, 8], F32)
                    wbig_sb = load(cst, "wbig", [128, NT, 128], BF16)
                    for f in range(1, 6):
                        nc.sync.dma_start(out=w6_sb[:, f, :], in_=w6.ap()[f])
                    tabs_sb = load(cst, "tabs", [64, S], BF16)
                    sinT0 = cst.tile([HALF, S], BF16)
                    nc.sync.dma_start(out=sinT0[:],
                                      in_=din["tabs"].ap()[HALF:ROPE, :])
                    hns = []
                    for i in range(NT):
                        hn = scrA.tile([128, HID], BF16, tag="hN", bufs=4)
                        nc.sync.dma_start(out=hn[:], in_=hN.ap()[i])
                        hns.append(hn)
                    ish_sb = load(cst, "ish", [128, 256], BF16)

                    # --- projections ---
                    for f in range(6):
                        ps = pq.tile([128, 2048], F32, name="p1")
                        for k in range(KT):
                            for sq in range(4):
                                nc.tensor.matmul(
                                    ps[:, sq * 512:(sq + 1) * 512],
                                    w6_sb[:, f, k * 128:(k + 1) * 128],
                                    hT_sb[:, k, sq * 512:(sq + 1) * 512],
                                    start=(k == 0), stop=(k == KT - 1))
                        if f < 4:
                            dst = qlkT[:, f, :]
                        else:
                            dst = lvT[:, f - 4, :]
                        bias = b6w_sb[:, f:f + 1]
                        nc.scalar.activation(dst[0:128, 0:1024], ps[:, 0:1024],
                                             AF.Identity, bias=bias)
                        nc.vector.tensor_scalar_add(dst[0:128, 1024:2048],
                                                    ps[:, 1024:2048], bias)
                        if f < 4:
                            nc.vector.tensor_mul(sqall[:, f, :], dst, dst)

                    # --- compressed entries (streams hN, interleaves) ---
                    ps_e = pe.tile([C, HID], F32)
                    for i in range(NT):
                        for hc in range(4):
                            nc.tensor.matmul(ps_e[:, hc * 512:(hc + 1) * 512],
                                             wbig_sb[:, i, :],
                                             hns[i][:, hc * 512:(hc + 1) * 512],
                                             start=(i == 0), stop=(i == NT - 1))
                    nc.scalar.copy(entries[:, 0:1024], ps_e[:, 0:1024])
                    nc.vector.tensor_copy(entries[:, 1024:2048],
                                          ps_e[:, 1024:2048])

                    # --- w-fold + rope (overlaps P1 on DVE; x2 half is
                    # shuttled to partitions 0:32 via SBUF->SBUF DMA) ---
                    cosT = tabs_sb[0:HALF, :]
                    for t4 in range(4):
                        wcol = b6w_sb[:, 6:7] if t4 < 2 else b6w_sb[:, 7:8]
                        nc.vector.tensor_scalar_mul(qlkT[:, t4, :],
                                                    qlkT[:, t4, :], wcol)
                        x1 = qlkT[0:HALF, t4, :]
                        x2d = scrA.tile([HALF, S], BF16, tag="x2d", bufs=1)
                        nc.sync.dma_start(out=x2d[:],
                                          in_=qlkT[HALF:ROPE, t4, :])
                        ta = scrA.tile([HALF, S], BF16, tag="ta", bufs=1)
                        tb = scrA.tile([HALF, S], BF16, tag="tb", bufs=1)
                        tc2 = scrA.tile([HALF, S], BF16, tag="tc2", bufs=1)
                        nc.vector.tensor_mul(ta[:], x1, cosT)
                        nc.vector.tensor_mul(tc2[:], x1, sinT0[:])
                        nc.vector.tensor_mul(tb[:], x2d[:], sinT0[:])
                        nc.vector.tensor_sub(x1, ta[:], tb[:])
                        nc.vector.tensor_mul(ta[:], x2d[:], cosT)
                        nc.vector.tensor_add(tb[:], tc2[:], ta[:])
                        nc.sync.dma_start(out=qlkT[HALF:ROPE, t4, :],
                                          in_=tb[:])

                # ---- mini: norms, k-fold, lv transpose, entries/kv ----
                with (
                    tc.tile_pool(name="scrM", bufs=2) as scrM,
                    tc.tile_pool(name="ps_ssq", bufs=1, space="PSUM") as pssq,
                    tc.tile_pool(name="ps_rsc", bufs=1, space="PSUM") as prsc,
                    tc.tile_pool(name="ps_tp", bufs=2, space="PSUM") as ptp,
                    tc.tile_pool(name="ps_bb", bufs=1, space="PSUM") as pbb,
                    tc.tile_pool(name="ps_kv", bufs=1, space="PSUM") as pkvp,
                ):
                    wkv_sb = load(scrM, "wkv", [128, KT, 256], BF16)
                    bkv_sb = load(scrM, "bkv", [1, 256], BF16)
                    ckro_sb = load(scrM, "ckro", [C, 192], F32)
                    sv_sb = load(scrM, "sv", [1, HPC * 128], BF16)
                    rsc_ps = prsc.tile([128, 64], F32)
                    for qtr in range(4):
                        ssq_ps = pssq.tile([4, 512], F32, name="ssq")
                        for t4 in range(4):
                            nc.tensor.matmul(
                                ssq_ps[:], sel_sb[:, 4 * t4:4 * t4 + 4],
                                sqall[:, t4, qtr * 512:(qtr + 1) * 512],
                                start=(t4 == 0), stop=(t4 == 3))
                        ssq_sb = scrM.tile([4, 512], F32, tag="ssqs")
                        nc.scalar.copy(ssq_sb[:], ssq_ps[:])
                        for j in range(4):
                            i = qtr * 4 + j
                            nc.tensor.matmul(
                                rsc_ps[:, 4 * i:4 * i + 4],
                                ssq_sb[0:4, j * 128:(j + 1) * 128],
                                ident_f32[0:4, 0:4], is_transpose=True,
                                start=(i == 0), stop=(i == NT - 1))
                    rms_sb = sts.tile([128, 64], F32)
                    nc.scalar.activation(rms_sb[:], rsc_ps[:], AF.Sqrt,
                                         scale=1.0 / HD, bias=eps_t[:])
                    nc.vector.reciprocal(rsc[:], rms_sb[:])
                    # fold softmax scale into the q columns only
                    rsc4 = rsc[:].rearrange("p (i t) -> p i t", t=4)
                    nc.vector.tensor_scalar_mul(rsc4[:, :, 0:2],
                                                rsc4[:, :, 0:2], SCALE)
                    # transpose the k columns, then flatten onto partition 0
                    rsck = sts.tile([128, 32], F32)
                    nc.vector.tensor_copy(
                        rsck[:].rearrange("p (i t) -> p i t", t=2),
                        rsc4[:, :, 2:4])
                    rT_ps = ptp.tile([128, 128], F32, tag="tp")
                    nc.tensor.matmul(rT_ps[0:32, :], rsck[:], ident_f32[:],
                                     is_transpose=True, start=True, stop=True)
                    nc.scalar.copy(rscT[:], rT_ps[0:32, :])
                    nc.sync.dma_start(out=rflat[:], in_=rscT[:])

                    # k-side 1/rms broadcast fold: lkT *= bcast(rsc_k)
                    for h in range(HPC):
                        bb_sb = scrM.tile([128, S], BF16, tag="bb", bufs=1)
                        for qtr in range(4):
                            bb_ps = pbb.tile([128, 512], F32, name="bb")
                            for j in range(4):
                                i = qtr * 4 + j
                                r = i * 2 + h
                                nc.tensor.matmul(
                                    bb_ps[:, j * 128:(j + 1) * 128],
                                    onesrow[:],
                                    rflat[:, r * 128:(r + 1) * 128],
                                    start=(j == 0), stop=(j == 3))
                            if qtr % 2 == 0:
                                nc.scalar.copy(
                                    bb_sb[:, qtr * 512:(qtr + 1) * 512],
                                    bb_ps[:])
                            else:
                                nc.vector.tensor_copy(
                                    bb_sb[:, qtr * 512:(qtr + 1) * 512],
                                    bb_ps[:])
                        nc.vector.tensor_mul(qlkT[:, 2 + h, :],
                                             qlkT[:, 2 + h, :], bb_sb[:])

                    # lv natural via PE transposes
                    for h in range(HPC):
                        for ti in range(NT):
                            tp = ptp.tile([128, 128], BF16, tag="tpl")
                            nc.tensor.matmul(
                                tp[:], lvT[:, h, ti * 128:(ti + 1) * 128],
                                ish_sb[:, 0:128], is_transpose=True,
                                start=True, stop=True)
                            if ti % 2 == 0:
                                nc.vector.tensor_copy(
                                    lvn[:, ti, h * 128:(h + 1) * 128], tp[:])
                            else:
                                nc.scalar.copy(
                                    lvn[:, ti, h * 128:(h + 1) * 128], tp[:])

                    # entries^T + shared ck/cv head
                    for k in range(KT):
                        tp = ptp.tile([128, 128], BF16, tag="tp")
                        nc.tensor.matmul(tp[:],
                                         entries[:, k * 128:(k + 1) * 128],
                                         ish_sb[:, 0:128], is_transpose=True,
                                         start=True, stop=True)
                        if k % 2 == 0:
                            nc.vector.tensor_copy(eT[:, k, :], tp[:])
                        else:
                            nc.scalar.copy(eT[:, k, :], tp[:])

                    ps_kv = pkvp.tile([C, 256], F32)
                    for k in range(KT):
                        nc.tensor.matmul(ps_kv[:], eT[:, k, :], wkv_sb[:, k, :],
                                         start=(k == 0), stop=False)
                    nc.tensor.matmul(ps_kv[:], onesrow[:], bkv_sb[:],
                                     start=False, stop=True)

                    # ck: rmsnorm + rope at block-end positions
                    sqc = scrM.tile([C, 128], F32, tag="sqc")
                    ssqc = sts.tile([C, 1], F32)
                    nc.scalar.activation(sqc[:], ps_kv[:, 0:128], AF.Square,
                                         accum_out=ssqc[:])
                    rmsc = sts.tile([C, 1], F32)
                    nc.scalar.activation(rmsc[:], ssqc[:], AF.Sqrt,
                                         scale=1.0 / HD, bias=eps_t[:])
                    rscc = sts.tile([C, 1], F32)
                    nc.vector.reciprocal(rscc[:], rmsc[:])
                    ckR = scrM.tile([C, 128], BF16, tag="ckR")
                    c1 = scrM.tile([C, HALF], F32, tag="ckt1")
                    c2 = scrM.tile([C, HALF], F32, tag="ckt2")
                    nc.vector.tensor_mul(c1[:], ps_kv[:, 0:HALF],
                                         ckro_sb[:, 0:32])
                    nc.vector.tensor_mul(c2[:], ps_kv[:, HALF:ROPE],
                                         ckro_sb[:, 32:64])
                    nc.vector.tensor_sub(ckR[:, 0:HALF], c1[:], c2[:])
                    nc.vector.tensor_mul(c1[:], ps_kv[:, 0:HALF],
                                         ckro_sb[:, 64:96])
                    nc.vector.tensor_mul(c2[:], ps_kv[:, HALF:ROPE],
                                         ckro_sb[:, 96:128])
                    nc.vector.tensor_add(ckR[:, HALF:ROPE], c1[:], c2[:])
                    nc.vector.tensor_mul(ckR[:, ROPE:128], ps_kv[:, ROPE:128],
                                         ckro_sb[:, 128:192])
                    nc.vector.tensor_scalar_mul(ckR[:], ckR[:], rscc[:])
                    tpc = ptp.tile([128, 128], BF16, tag="tp")
                    nc.tensor.matmul(tpc[:], ckR[:], ish_sb[:, 0:128],
                                     is_transpose=True, start=True, stop=True)
                    nc.vector.tensor_copy(ckT[:], tpc[:])

                    nc.scalar.copy(cvn[:], ps_kv[:, 128:256])
                    nc.sync.dma_start(out=cv127[:], in_=cvn[127:128, :])
                    for h in range(HPC):
                        tps = ptp.tile([128, 128], F32, tag="tp")
                        nc.tensor.matmul(tps[:], ish_sb[:, 128:256], cvn[:],
                                         start=True, stop=False)
                        nc.tensor.matmul(tps[:], ish_sb[0:1, 0:128],
                                         sv_sb[0:1, h * 128:(h + 1) * 128],
                                         start=False, stop=True)
                        nc.scalar.copy(sinkcv[:, h, :], tps[:])

            # ============ late phases (P3 + P4) ==========================
            with (
                tc.tile_pool(name="late", bufs=1) as late,
                tc.tile_pool(name="scrB", bufs=2) as scrB,
                tc.tile_pool(name="ps_sc", bufs=2, space="PSUM") as psc,
                tc.tile_pool(name="ps_pt", bufs=2, space="PSUM") as pptp,
                tc.tile_pool(name="ps_cx", bufs=2, space="PSUM") as pcx,
                tc.tile_pool(name="ps_o", bufs=2, space="PSUM") as pout,
            ):
                mgT = late.tile([128, HPC, S], BF16)   # merged^T
                loc01_sb = load(late, "loc01", [128, 256], BF16)
                cm_sb = load(late, "cm01", [128, NT, 129], BF16)
                skT_sb = load(late, "skT", [128, HPC], BF16)
                wo_sb = load(late, "wo", [128, HPC, HID], BF16)

                for i in range(NT):
                    lo = 0 if i > 0 else 128
                    den = sts.tile([128, 4], F32)
                    rden = sts.tile([128, 4], F32)
                    p_sb = scrB.tile([128, HPC, 385], BF16, tag="p", bufs=3)
                    for h in range(HPC):
                        ps_s = psc.tile([128, 385], F32, name="sc")
                        qT = qlkT[:, h, i * 128:(i + 1) * 128]
                        if i > 0:
                            nc.tensor.matmul(
                                ps_s[:, 0:256], qT,
                                qlkT[:, 2 + h, (i - 1) * 128:(i + 1) * 128],
                                start=True, stop=True)
                        else:
                            nc.tensor.matmul(ps_s[:, 128:256], qT,
                                             qlkT[:, 2 + h, 0:128],
                                             start=True, stop=True)
                        nc.tensor.matmul(ps_s[:, 256:257], qT,
                                         skT_sb[:, h:h + 1],
                                         start=True, stop=True)
                        nc.tensor.matmul(ps_s[:, 257:385], qT, ckT[:],
                                         start=True, stop=True)
                        nc.scalar.activation(
                            p_sb[:, h, lo:385], ps_s[:, lo:385], AF.Exp,
                            scale=rsc[:, 4 * i + h:4 * i + h + 1])
                    locb = (loc01_sb[:, lo:256].unsqueeze(1)
                            .broadcast_to([128, HPC, 256 - lo]))
                    nc.gpsimd.tensor_mul(p_sb[:, :, lo:256],
                                         p_sb[:, :, lo:256], locb)
                    cmb = (cm_sb[:, i, :].unsqueeze(1)
                           .broadcast_to([128, HPC, 129]))
                    nc.gpsimd.tensor_mul(p_sb[:, :, 256:385],
                                         p_sb[:, :, 256:385], cmb)
                    # den cols: [local h0, local h1, comp h0, comp h1]
                    nc.vector.tensor_reduce(den[:, 0:2], p_sb[:, :, lo:256],
                                            mybir.AxisListType.X, ALU.add)
                    nc.vector.tensor_reduce(den[:, 2:4], p_sb[:, :, 256:385],
                                            mybir.AxisListType.X, ALU.add)
                    nc.vector.reciprocal(rden[:], den[:])
                    for h in range(HPC):
                        ctx = pcx.tile([128, 128], F32, name="cx")
                        lo2 = 0 if i > 0 else 128
                        nc.gpsimd.tensor_scalar(
                            out=p_sb[:, h, lo2:256],
                            in0=p_sb[:, h, lo2:256],
                            scalar1=rden[:, h:h + 1], scalar2=0.5,
                            op0=ALU.mult, op1=ALU.mult)
                        nc.vector.tensor_scalar(
                            out=p_sb[:, h, 256:385],
                            in0=p_sb[:, h, 256:385],
                            scalar1=rden[:, 2 + h:3 + h],
                            scalar2=0.5, op0=ALU.mult, op1=ALU.mult)
                        pt = pptp.tile([128, 4, 128], BF16, name="pt")
                        last15 = (i == NT - 1)
                        if i > 0:
                            nc.tensor.matmul(pt[:, 0, :], p_sb[:, h, 0:128],
                                             ish_sb[:, 0:128],
                                             is_transpose=True,
                                             start=True, stop=False)
                        nc.tensor.matmul(pt[:, 1, :], p_sb[:, h, 128:256],
                                         ish_sb[:, 0:128],
                                         is_transpose=True,
                                         start=(i == 0), stop=False)
                        if last15:
                            nc.tensor.matmul(pt[0:1, 3, :],
                                             p_sb[:, h, 384:385],
                                             ish_sb[:, 0:128],
                                             is_transpose=True,
                                             start=False, stop=False)
                        nc.tensor.matmul(pt[:, 2, :], p_sb[:, h, 256:384],
                                         ish_sb[:, 0:128],
                                         is_transpose=True,
                                         start=False, stop=True)
                        ptsb = scrB.tile([128, 4, 128], BF16, tag="pt", bufs=3)
                        c0 = 0 if i > 0 else 1
                        if h == 0:
                            nc.scalar.copy(ptsb[:, c0:3, :], pt[:, c0:3, :])
                        else:
                            nc.vector.tensor_copy(ptsb[:, c0:3, :],
                                                  pt[:, c0:3, :])
                        if last15:
                            nc.vector.tensor_copy(ptsb[0:1, 3, :],
                                                  pt[0:1, 3, :])
                        if i > 0:
                            nc.tensor.matmul(
                                ctx[:], lvn[:, i - 1, h * 128:(h + 1) * 128],
                                ptsb[:, 0, :], start=True, stop=False)
                        nc.tensor.matmul(ctx[:],
                                         lvn[:, i, h * 128:(h + 1) * 128],
                                         ptsb[:, 1, :], start=(i == 0),
                                         stop=False)
                        nc.tensor.matmul(ctx[:], sinkcv[:, h, :],
                                         ptsb[:, 2, :], start=False,
                                         stop=not last15)
                        if last15:
                            nc.tensor.matmul(ctx[:], cv127[:],
                                             ptsb[0:1, 3, :], start=False,
                                             stop=True)
                        if h == 0:
                            nc.scalar.copy(mgT[:, h, i * 128:(i + 1) * 128],
                                           ctx[:])
                        else:
                            nc.vector.tensor_copy(
                                mgT[:, h, i * 128:(i + 1) * 128], ctx[:])

                    # ---- P4 for this s-tile ----
                    o_sb = scrB.tile([128, HID], BF16, tag="o", bufs=3)
                    for qo in range(4):
                        po = pout.tile([128, 512], F32, name="o4")
                        for h in range(HPC):
                            nc.tensor.matmul(
                                po[:], mgT[:, h, i * 128:(i + 1) * 128],
                                wo_sb[:, h, qo * 512:(qo + 1) * 512],
                                start=(h == 0), stop=(h == HPC - 1))
                        # 1024 cols on ACT, 1024 on DVE (engine balance)
                        off = qo * 512
                        if qo < 2:
                            nc.scalar.copy(o_sb[:, off:off + 512], po[:])
                        else:
                            nc.vector.tensor_copy(o_sb[:, off:off + 512],
                                                  po[:])
                    nc.sync.dma_start(out=out_p.ap()[i], in_=o_sb[:])

    nc.compile()
    return nc


def _host_prep(inputs):
    """Build the 8 per-core input maps from full inputs."""
    hs = np.asarray(inputs["hidden_states"], np.float32)[0]  # [S, HID]
    Wq = np.asarray(inputs["Wq"], np.float32)
    Wc = np.asarray(inputs["Wc"], np.float32)
    Wk = np.asarray(inputs["Wk"], np.float32)
    Wv = np.asarray(inputs["Wv"], np.float32)
    Wlk = np.asarray(inputs["Wlk"], np.float32)
    Wlv = np.asarray(inputs["Wlv"], np.float32)
    qn_w = np.asarray(inputs["qn_w"], np.float32)
    kn_w = np.asarray(inputs["kn_w"], np.float32)
    sink_k = np.asarray(inputs["sink_k"], np.float32)
    sink_v = np.asarray(inputs["sink_v"], np.float32)
    Wo = np.asarray(inputs["Wo"], np.float32)
    bq = np.asarray(inputs["bq"], np.float32)
    bc = np.asarray(inputs["bc"], np.float32)
    bk = np.asarray(inputs["bk"], np.float32)
    bv = np.asarray(inputs["bv"], np.float32)
    blk = np.asarray(inputs["blk"], np.float32)
    blv = np.asarray(inputs["blv"], np.float32)

    hT_t = np.ascontiguousarray(hs.T.reshape(KT, 128, S)).astype(NPBF)
    hN_t = hs.reshape(NT, 128, HID).astype(NPBF)

    # host-side compressor softmax (0.016% of FLOPs)
    cw = hs @ Wc[:, 0] + bc[0]                      # [S]
    cwb = cw.reshape(C, R)
    e = np.exp(cwb - cwb.max(axis=1, keepdims=True))
    w = e / e.sum(axis=1, keepdims=True)            # [C, R]
    s_idx = np.arange(S)
    wbig = np.zeros((128, NT, 128), np.float32)
    wbig[s_idx % 128, s_idx // 128, s_idx // R] = w[s_idx // R, s_idx % R]
    wbig = wbig.reshape(128, NT * 128).astype(NPBF)

    # rope tables [freq, pos]
    pos = np.arange(S, dtype=np.float32)
    inv_freq = 1.0 / (10000.0 ** (np.arange(HALF, dtype=np.float32) * 2.0 / ROPE))
    ang = inv_freq[:, None] * pos[None, :]          # [HALF, S]
    tabs = np.concatenate([np.cos(ang), np.sin(ang)], axis=0).astype(NPBF)

    # ck rope tables at block-end positions (kn_w folded)
    pos_c = (np.arange(C, dtype=np.float32) * R + (R - 1))
    angc = pos_c[:, None] * inv_freq[None, :]       # [C, HALF]
    cosc, sinc = np.cos(angc), np.sin(angc)
    kw1, kw2 = kn_w[0:HALF], kn_w[HALF:ROPE]
    ckro = np.concatenate([
        cosc * kw1[None, :], sinc * kw2[None, :],
        sinc * kw1[None, :], cosc * kw2[None, :],
        np.broadcast_to(kn_w[ROPE:][None, :], (C, ROPE)),
    ], axis=1).astype(np.float32)

    # identity | shift(+1)
    r = np.arange(128)
    ident = (r[:, None] == r[None, :]).astype(np.float32)
    shift = (r[None, :] == r[:, None] + 1).astype(np.float32)
    ish = np.concatenate([ident, shift], axis=1).astype(NPBF)

    # masks (multiplicative 0/1)
    p = r[:, None]
    j = np.arange(256)[None, :]
    loc01 = np.where(j < 128, (j >= p), (j - 128 <= p)).astype(np.float32)
    loc01 = loc01.astype(NPBF)
    cm = np.zeros((128, NT, 129), np.float32)
    cm[:, :, 0] = 1.0
    ii = np.arange(NT)[None, :, None]
    cc = np.arange(128)[None, None, :]
    cm[:, :, 1:] = (cc * R + (R - 1) <= ii * 128 + p[:, None]).astype(np.float32)
    cm = cm.reshape(128, NT * 129).astype(NPBF)

    wkv_h = np.ascontiguousarray(
        np.concatenate([Wk, Wv], axis=1).reshape(KT, 128, 256)
        .transpose(1, 0, 2)).reshape(128, KT * 256).astype(NPBF)

    common = dict(hT=hT_t, hN=hN_t, tabs=tabs, wbig=wbig, wkv=wkv_h,
                  bkv=np.concatenate([bk, bv])[None, :].astype(NPBF),
                  ckro=ckro, ish=ish, loc01=loc01, cm01=cm)

    Wq4 = Wq.reshape(HID, NH, HD)
    Wlk4 = Wlk.reshape(HID, NH, HD)
    Wlv4 = Wlv.reshape(HID, NH, HD)
    bq4 = bq.reshape(NH, HD)
    blk4 = blk.reshape(NH, HD)
    blv4 = blv.reshape(NH, HD)
    Wo4 = Wo.reshape(NH, HD, HID)

    def tchunk(wcol):  # [HID, 128] -> [128, 2048] transposed-chunk layout
        return np.ascontiguousarray(
            wcol.reshape(KT, 128, 128).transpose(1, 0, 2)).reshape(128, 2048)

    in_maps = []
    for c in range(NCORES):
        hh = [HPC * c + h for h in range(HPC)]
        w6 = np.stack([
            tchunk(Wq4[:, hh[0]]), tchunk(Wq4[:, hh[1]]),
            tchunk(Wlk4[:, hh[0]]), tchunk(Wlk4[:, hh[1]]),
            tchunk(Wlv4[:, hh[0]]), tchunk(Wlv4[:, hh[1]]),
        ]).astype(NPBF)
        b6w = np.stack([bq4[hh[0]], bq4[hh[1]], blk4[hh[0]], blk4[hh[1]],
                        blv4[hh[0]], blv4[hh[1]], qn_w, kn_w],
                       axis=1).astype(np.float32)
        wo_c = np.ascontiguousarray(
            Wo4[hh].transpose(1, 0, 2)).reshape(128, HPC * HID).astype(NPBF)
        m = dict(common)
        m.update(w6=w6, b6w=b6w, wo=wo_c,
                 skT=sink_k[hh].T.astype(NPBF),
                 sv=sink_v[hh].astype(NPBF))
        in_maps.append(m)
    return in_maps


def kernel(**inputs):
    if "nc" not in _CACHE:
        _CACHE["nc"] = _build_bass()
    nc = _CACHE["nc"]
    in_maps = _host_prep(inputs)
    res = run_bass_kernel_spmd(nc, in_maps, core_ids=list(range(NCORES)))
    out = np.zeros((S, HID), np.float64)
    for c in range(NCORES):
        out += res.results[c]["out_p"].reshape(S, HID).astype(np.float64)
    out += np.asarray(inputs["bo"], np.float32)[None, :]
    return out[None].astype(np.float32)


# revision 34
# speedup vs baseline: 1.0175x; 1.0015x over previous
"""Trainium2 Bass kernel for HeavilyCompressedAttention.

Sharding: 16 heads across 8 cores (2 heads/core, tensor-parallel);
compressed-KV path (single shared head) replicated on every core;
out_proj row-parallel with host-side partial sum (bf16 partials).

Design notes (cost-model driven):
  - Projections run weight-stationary producing TRANSPOSED outputs
    (qT/lkT/lvT = [d, s]) -- exactly the stationary layout the attention
    scores need, so there are no per-tile q/k transposes.
  - The compressed-entries matmuls, the raw squares, the w-fold and the
    RoPE rotation all overlap the projection phase (PSUM split 4+4
    banks; hN streamed behind hT in the in-order DMA queue).
  - RMSNorm is algebraic-folded: ssq reduced via PE selector matmuls +
    tiny transposes; 1/rms applied as a per-partition activation scale
    on the q side and a PE-outer-product broadcast fold on the k side.
  - Softmax: unmasked exp (logits are O(1) by construction), 0/1
    mask-multiplies on the GpSimd engine, free-dim reduces for the
    denominators, 1/den and the 0.5 branch weight folded into one
    two-scalar tensor_scalar.
  - Compressor softmax weights are host-prepared (0.016% of FLOPs).
  - Few, large DMAs; bf16 output partials summed on host.
"""

import os
import sys

import numpy as np
import ml_dtypes

for _p in ("/opt/trn_rl_repo", "/root/.axon_site/_ro/trn_rl_repo"):
    if os.path.isdir(_p) and _p not in sys.path:
        sys.path.insert(0, _p)

from concourse import bacc, mybir  # noqa: E402
import concourse.tile as tile  # noqa: E402
from concourse.bass_utils import run_bass_kernel_spmd  # noqa: E402
from concourse.masks import make_identity  # noqa: E402

F32 = mybir.dt.float32
BF16 = mybir.dt.bfloat16
NPBF = ml_dtypes.bfloat16
AF = mybir.ActivationFunctionType
ALU = mybir.AluOpType

S = 2048
HID = 2048
NH = 16
HD = 128
R = 16
C = S // R  # 128
WIN = 128
ROPE = HD // 2  # 64
HALF = ROPE // 2  # 32
EPS = 1e-6
NT = S // 128  # 16 s-tiles
KT = HID // 128  # 16 k-tiles
NCORES = 8
HPC = NH // NCORES  # 2 heads per core
SCALE = 1.0 / float(np.sqrt(HD))

_CACHE = {}


def _build_bass():
    nc = bacc.Bacc("TRN2", target_bir_lowering=False, debug=False,
                   num_devices=NCORES)

    din = {}

    def inp(name, shape, dt):
        din[name] = nc.dram_tensor(name, list(shape), dt, kind="ExternalInput")
        return din[name]

    hT = inp("hT", [KT, 128, S], BF16)        # hidden^T chunks [k][d, s]
    hN = inp("hN", [NT, 128, HID], BF16)      # hidden natural s-tiles
    w6 = inp("w6", [6, 128, 2048], BF16)      # [q0|q1|lk0|lk1|lv0|lv1] chunks
    b6w = inp("b6w", [128, 8], F32)           # biases (6 cols) | qn_w | kn_w
    tabs = inp("tabs", [64, S], BF16)         # rope tables [cos;sin][freq, pos]
    wbig = inp("wbig", [128, NT * 128], BF16)  # host-softmaxed compressor wts
    wkv = inp("wkv", [128, KT * 256], BF16)   # [Wk|Wv] shared head
    bkv = inp("bkv", [1, 256], BF16)
    wo = inp("wo", [128, HPC * HID], BF16)    # Wo rows per head [d][h, o]
    ckro = inp("ckro", [C, 192], F32)         # ck rope tabs A|B|C|D|pass
    ish = inp("ish", [128, 256], BF16)        # identity | shift(+1)
    loc01 = inp("loc01", [128, 256], BF16)    # local window 0/1 mask
    cm01 = inp("cm01", [128, NT * 129], BF16)  # compressed 0/1 mask (+sink col)
    skT = inp("skT", [128, HPC], BF16)        # sink_k columns
    sv = inp("sv", [HPC, 128], BF16)          # sink_v rows

    out_p = nc.dram_tensor("out_p", [NT, 128, HID], BF16, kind="ExternalOutput")

    with tile.TileContext(nc) as tc:
        with (
            tc.tile_pool(name="const", bufs=1) as cst,
            tc.tile_pool(name="persist", bufs=1) as per,
            tc.tile_pool(name="stats", bufs=4) as sts,
        ):
            # ---------------- consts ----------------
            def load(pool, name, shape, dt):
                t = pool.tile(list(shape), dt, name=f"c_{name}")
                nc.sync.dma_start(out=t[:], in_=din[name].ap())
                return t

            ident_f32 = cst.tile([128, 128], F32)
            make_identity(nc, ident_f32[:])
            onesrow = cst.tile([1, 128], BF16)
            nc.vector.memset(onesrow[:], 1.0)
            eps_t = cst.tile([128, 1], F32)
            nc.vector.memset(eps_t[:], EPS)
            sel_sb = cst.tile([128, 16], BF16)
            nc.vector.memset(sel_sb[:], 0.0)
            for t4 in range(4):
                nc.vector.memset(sel_sb[:, 5 * t4:5 * t4 + 1], 1.0)

            # ---------------- persistent activations ----------------
            qlkT = per.tile([128, 4, S], BF16)     # roped q|lk transposed
            lvn = per.tile([128, NT, 256], BF16)   # lv natural
            rsc = per.tile([128, 64], F32)         # 1/rms  [(i,tensor)]
            rflat = per.tile([1, 32 * 128], BF16)  # k-col 1/rms on partition 0
            entries = per.tile([C, HID], BF16)
            eT = per.tile([128, KT, C], BF16)
            cvn = per.tile([C, 128], BF16)
            sinkcv = per.tile([128, HPC, 128], BF16)
            ckT = per.tile([128, C], BF16)
            cv127 = per.tile([1, 128], BF16)

            # ====== P1 (+ entries, squares, rope, all overlapped) =======
            with tc.tile_pool(name="mid", bufs=1) as mid:
                lvT = mid.tile([128, HPC, S], BF16)   # lv transposed
                sqall = mid.tile([128, 4, S], BF16)   # raw squares for ssq
                rscT = mid.tile([32, 128], BF16)      # k-cols of 1/rms, transp
                with (
                    tc.tile_pool(name="scrA", bufs=2) as scrA,
                    tc.tile_pool(name="bulk", bufs=1) as blk,
                    tc.tile_pool(name="ps_p1", bufs=1, space="PSUM") as pq,
                    tc.tile_pool(name="ps_e", bufs=1, space="PSUM") as pe,
                ):
                    w6_sb = blk.tile([128, 6, 2048], BF16)
                    hT_sb = blk.tile([128, KT, S], BF16)
                    # DMA issue order matters (SP queue is in-order):
                    # w6[0], wbig, hT chunks, w6 rest, early consts, hN ring
                    nc.sync.dma_start(out=w6_sb[:, 0, :], in_=w6.ap()[0])
                    for k in range(KT):
                        nc.sync.dma_start(out=hT_sb[:, k, :], in_=hT.ap()[k])
                    b6w_sb = load(cst, "b6w", [128, 8], F32)
                    wbig_sb = load(cst, "wbig", [128, NT, 128], BF16)
                    for f in range(1, 6):
                        nc.sync.dma_start(out=w6_sb[:, f, :], in_=w6.ap()[f])
                    tabs_sb = load(cst, "tabs", [64, S], BF16)
                    sinT0 = cst.tile([HALF, S], BF16)
                    nc.sync.dma_start(out=sinT0[:],
                                      in_=din["tabs"].ap()[HALF:ROPE, :])
                    hns = []
                    for i in range(NT):
                        hn = scrA.tile([128, HID], BF16, tag="hN", bufs=4)
                        nc.sync.dma_start(out=hn[:], in_=hN.ap()[i])
                        hns.append(hn)
                    ish_sb = load(cst, "ish", [128, 256], BF16)

                    # --- projections ---
                    for f in range(6):
                        ps = pq.tile([128, 2048], F32, name="p1")
                        for k in range(KT):
                            for sq in range(4):
                                nc.tensor.matmul(
                                    ps[:, sq * 512:(sq + 1) * 512],
                                    w6_sb[:, f, k * 128:(k + 1) * 128],
                                    hT_sb[:, k, sq * 512:(sq + 1) * 512],
                                    start=(k == 0), stop=(k == KT - 1))
                        if f < 4:
                            dst = qlkT[:, f, :]
                        else:
                            dst = lvT[:, f - 4, :]
                        bias = b6w_sb[:, f:f + 1]
                        nc.scalar.activation(dst[0:128, 0:1024], ps[:, 0:1024],
                                             AF.Identity, bias=bias)
                        nc.vector.tensor_scalar_add(dst[0:128, 1024:2048],
                                                    ps[:, 1024:2048], bias)
                        if f < 4:
                            nc.vector.tensor_mul(sqall[:, f, :], dst, dst)

                    # --- compressed entries (streams hN, interleaves) ---
                    ps_e = pe.tile([C, HID], F32)
                    for i in range(NT):
                        for hc in range(4):
                            nc.tensor.matmul(ps_e[:, hc * 512:(hc + 1) * 512],
                                             wbig_sb[:, i, :],
                                             hns[i][:, hc * 512:(hc + 1) * 512],
                                             start=(i == 0), stop=(i == NT - 1))
                    nc.scalar.copy(entries[:, 0:1024], ps_e[:, 0:1024])
                    nc.vector.tensor_copy(entries[:, 1024:2048],
                                          ps_e[:, 1024:2048])

                    # --- w-fold + rope (overlaps P1 on DVE; x2 half is
                    # shuttled to partitions 0:32 via SBUF->SBUF DMA) ---
                    cosT = tabs_sb[0:HALF, :]
                    for t4 in range(4):
                        wcol = b6w_sb[:, 6:7] if t4 < 2 else b6w_sb[:, 7:8]
                        nc.vector.tensor_scalar_mul(qlkT[:, t4, :],
                                                    qlkT[:, t4, :], wcol)
                        x1 = qlkT[0:HALF, t4, :]
                        x2d = scrA.tile([HALF, S], BF16, tag="x2d", bufs=1)
                        nc.sync.dma_start(out=x2d[:],
                                          in_=qlkT[HALF:ROPE, t4, :])
                        ta = scrA.tile([HALF, S], BF16, tag="ta", bufs=1)
                        tb = scrA.tile([HALF, S], BF16, tag="tb", bufs=1)
                        tc2 = scrA.tile([HALF, S], BF16, tag="tc2", bufs=1)
                        nc.vector.tensor_mul(ta[:], x1, cosT)
                        nc.vector.tensor_mul(tc2[:], x1, sinT0[:])
                        nc.vector.tensor_mul(tb[:], x2d[:], sinT0[:])
                        nc.vector.tensor_sub(x1, ta[:], tb[:])
                        nc.vector.tensor_mul(ta[:], x2d[:], cosT)
                        nc.vector.tensor_add(tb[:], tc2[:], ta[:])
                        nc.sync.dma_start(out=qlkT[HALF:ROPE, t4, :],
                                          in_=tb[:])

                # ---- mini: norms, k-fold, lv transpose, entries/kv ----
                with (
                    tc.tile_pool(name="scrM", bufs=2) as scrM,
                    tc.tile_pool(name="ps_ssq", bufs=1, space="PSUM") as pssq,
                    tc.tile_pool(name="ps_rsc", bufs=1, space="PSUM") as prsc,
                    tc.tile_pool(name="ps_tp", bufs=2, space="PSUM") as ptp,
                    tc.tile_pool(name="ps_kv", bufs=1, space="PSUM") as pkvp,
                ):
                    wkv_sb = load(scrM, "wkv", [128, KT, 256], BF16)
                    bkv_sb = load(scrM, "bkv", [1, 256], BF16)
                    ckro_sb = load(scrM, "ckro", [C, 192], F32)
                    sv_sb = load(scrM, "sv", [1, HPC * 128], BF16)
                    rsc_ps = prsc.tile([128, 64], F32)
                    for qtr in range(4):
                        ssq_ps = pssq.tile([4, 512], F32, name="ssq")
                        for t4 in range(4):
                            nc.tensor.matmul(
                                ssq_ps[:], sel_sb[:, 4 * t4:4 * t4 + 4],
                                sqall[:, t4, qtr * 512:(qtr + 1) * 512],
                                start=(t4 == 0), stop=(t4 == 3))
                        ssq_sb = scrM.tile([4, 512], F32, tag="ssqs")
                        nc.scalar.copy(ssq_sb[:], ssq_ps[:])
                        for j in range(4):
                            i = qtr * 4 + j
                            nc.tensor.matmul(
                                rsc_ps[:, 4 * i:4 * i + 4],
                                ssq_sb[0:4, j * 128:(j + 1) * 128],
                                ident_f32[0:4, 0:4], is_transpose=True,
                                start=(i == 0), stop=(i == NT - 1))
                    rms_sb = sts.tile([128, 64], F32)
                    nc.scalar.activation(rms_sb[:], rsc_ps[:], AF.Sqrt,
                                         scale=1.0 / HD, bias=eps_t[:])
                    nc.vector.reciprocal(rsc[:], rms_sb[:])
                    # fold softmax scale into the q columns only
                    rsc4 = rsc[:].rearrange("p (i t) -> p i t", t=4)
                    nc.vector.tensor_scalar_mul(rsc4[:, :, 0:2],
                                                rsc4[:, :, 0:2], SCALE)
                    # transpose the k columns, then flatten onto partition 0
                    rsck = sts.tile([128, 32], F32)
                    nc.vector.tensor_copy(
                        rsck[:].rearrange("p (i t) -> p i t", t=2),
                        rsc4[:, :, 2:4])
                    rT_ps = ptp.tile([128, 128], F32, tag="tp")
                    nc.tensor.matmul(rT_ps[0:32, :], rsck[:], ident_f32[:],
                                     is_transpose=True, start=True, stop=True)
                    nc.scalar.copy(rscT[:], rT_ps[0:32, :])
                    nc.sync.dma_start(out=rflat[:], in_=rscT[:])

                    # k-side 1/rms broadcast fold: lkT *= bcast(rsc_k)
                    for h in range(HPC):
                        bb_sb = scrM.tile([128, S], BF16, tag="bb", bufs=1)
                        for qtr in range(4):
                            bb_ps = pssq.tile([128, 512], F32, name="bb", tag="")
                            for j in range(4):
                                i = qtr * 4 + j
                                r = i * 2 + h
                                nc.tensor.matmul(
                                    bb_ps[:, j * 128:(j + 1) * 128],
                                    onesrow[:],
                                    rflat[:, r * 128:(r + 1) * 128],
                                    start=(j == 0), stop=(j == 3))
                            nc.vector.tensor_copy(
                                bb_sb[:, qtr * 512:(qtr + 1) * 512],
                                bb_ps[:])
                        nc.vector.tensor_mul(qlkT[:, 2 + h, :],
                                             qlkT[:, 2 + h, :], bb_sb[:])

                    # lv natural via PE transposes
                    for h in range(HPC):
                        for ti in range(NT):
                            tp = ptp.tile([128, 128], BF16, tag="tpl")
                            nc.tensor.matmul(
                                tp[:], lvT[:, h, ti * 128:(ti + 1) * 128],
                                ish_sb[:, 0:128], is_transpose=True,
                                start=True, stop=True)
                            if ti % 2 == 0:
                                nc.vector.tensor_copy(
                                    lvn[:, ti, h * 128:(h + 1) * 128], tp[:])
                            else:
                                nc.scalar.copy(
                                    lvn[:, ti, h * 128:(h + 1) * 128], tp[:])

                    # entries^T + shared ck/cv head
                    for k in range(KT):
                        tp = ptp.tile([128, 128], BF16, tag="tp")
                        nc.tensor.matmul(tp[:],
                                         entries[:, k * 128:(k + 1) * 128],
                                         ish_sb[:, 0:128], is_transpose=True,
                                         start=True, stop=True)
                        if k % 2 == 0:
                            nc.vector.tensor_copy(eT[:, k, :], tp[:])
                        else:
                            nc.scalar.copy(eT[:, k, :], tp[:])

                    ps_kv = pkvp.tile([C, 256], F32)
                    for k in range(KT):
                        nc.tensor.matmul(ps_kv[:], eT[:, k, :], wkv_sb[:, k, :],
                                         start=(k == 0), stop=False)
                    nc.tensor.matmul(ps_kv[:], onesrow[:], bkv_sb[:],
                                     start=False, stop=True)

                    # ck: rmsnorm + rope at block-end positions
                    sqc = scrM.tile([C, 128], F32, tag="sqc")
                    ssqc = sts.tile([C, 1], F32)
                    nc.scalar.activation(sqc[:], ps_kv[:, 0:128], AF.Square,
                                         accum_out=ssqc[:])
                    rmsc = sts.tile([C, 1], F32)
                    nc.scalar.activation(rmsc[:], ssqc[:], AF.Sqrt,
                                         scale=1.0 / HD, bias=eps_t[:])
                    rscc = sts.tile([C, 1], F32)
                    nc.vector.reciprocal(rscc[:], rmsc[:])
                    ckR = scrM.tile([C, 128], BF16, tag="ckR")
                    c1 = scrM.tile([C, HALF], F32, tag="ckt1")
                    c2 = scrM.tile([C, HALF], F32, tag="ckt2")
                    nc.vector.tensor_mul(c1[:], ps_kv[:, 0:HALF],
                                         ckro_sb[:, 0:32])
                    nc.vector.tensor_mul(c2[:], ps_kv[:, HALF:ROPE],
                                         ckro_sb[:, 32:64])
                    nc.vector.tensor_sub(ckR[:, 0:HALF], c1[:], c2[:])
                    nc.vector.tensor_mul(c1[:], ps_kv[:, 0:HALF],
                                         ckro_sb[:, 64:96])
                    nc.vector.tensor_mul(c2[:], ps_kv[:, HALF:ROPE],
                                         ckro_sb[:, 96:128])
                    nc.vector.tensor_add(ckR[:, HALF:ROPE], c1[:], c2[:])
                    nc.vector.tensor_mul(ckR[:, ROPE:128], ps_kv[:, ROPE:128],
                                         ckro_sb[:, 128:192])
                    nc.vector.tensor_scalar_mul(ckR[:], ckR[:], rscc[:])
                    tpc = ptp.tile([128, 128], BF16, tag="tp")
                    nc.tensor.matmul(tpc[:], ckR[:], ish_sb[:, 0:128],
                                     is_transpose=True, start=True, stop=True)
                    nc.vector.tensor_copy(ckT[:], tpc[:])

                    nc.scalar.copy(cvn[:], ps_kv[:, 128:256])
                    nc.sync.dma_start(out=cv127[:], in_=cvn[127:128, :])
                    for h in range(HPC):
                        tps = ptp.tile([128, 128], F32, tag="tp")
                        nc.tensor.matmul(tps[:], ish_sb[:, 128:256], cvn[:],
                                         start=True, stop=False)
                        nc.tensor.matmul(tps[:], ish_sb[0:1, 0:128],
                                         sv_sb[0:1, h * 128:(h + 1) * 128],
                                         start=False, stop=True)
                        nc.scalar.copy(sinkcv[:, h, :], tps[:])

            # ============ late phases (P3 + P4) ==========================
            with (
                tc.tile_pool(name="late", bufs=1) as late,
                tc.tile_pool(name="scrB", bufs=2) as scrB,
                tc.tile_pool(name="ps_sc", bufs=2, space="PSUM") as psc,
                tc.tile_pool(name="ps_pt", bufs=2, space="PSUM") as pptp,
                tc.tile_pool(name="ps_cx", bufs=2, space="PSUM") as pcx,
                tc.tile_pool(name="ps_o", bufs=2, space="PSUM") as pout,
            ):
                mgT = late.tile([128, HPC, S], BF16)   # merged^T
                loc01_sb = load(late, "loc01", [128, 256], BF16)
                cm_sb = load(late, "cm01", [128, NT, 129], BF16)
                skT_sb = load(late, "skT", [128, HPC], BF16)
                wo_sb = load(late, "wo", [128, HPC, HID], BF16)

                for i in range(NT):
                    lo = 0 if i > 0 else 128
                    den = sts.tile([128, 4], F32)
                    rden = sts.tile([128, 4], F32)
                    p_sb = scrB.tile([128, HPC, 385], BF16, tag="p", bufs=3)
                    for h in range(HPC):
                        ps_s = psc.tile([128, 385], F32, name="sc")
                        qT = qlkT[:, h, i * 128:(i + 1) * 128]
                        if i > 0:
                            nc.tensor.matmul(
                                ps_s[:, 0:256], qT,
                                qlkT[:, 2 + h, (i - 1) * 128:(i + 1) * 128],
                                start=True, stop=True)
                        else:
                            nc.tensor.matmul(ps_s[:, 128:256], qT,
                                             qlkT[:, 2 + h, 0:128],
                                             start=True, stop=True)
                        nc.tensor.matmul(ps_s[:, 256:257], qT,
                                         skT_sb[:, h:h + 1],
                                         start=True, stop=True)
                        nc.tensor.matmul(ps_s[:, 257:385], qT, ckT[:],
                                         start=True, stop=True)
                        nc.scalar.activation(
                            p_sb[:, h, lo:385], ps_s[:, lo:385], AF.Exp,
                            scale=rsc[:, 4 * i + h:4 * i + h + 1])
                    locb = (loc01_sb[:, lo:256].unsqueeze(1)
                            .broadcast_to([128, HPC, 256 - lo]))
                    nc.gpsimd.tensor_mul(p_sb[:, :, lo:256],
                                         p_sb[:, :, lo:256], locb)
                    cmb = (cm_sb[:, i, :].unsqueeze(1)
                           .broadcast_to([128, HPC, 129]))
                    nc.gpsimd.tensor_mul(p_sb[:, :, 256:385],
                                         p_sb[:, :, 256:385], cmb)
                    # den cols: [local h0, local h1, comp h0, comp h1]
                    nc.vector.tensor_reduce(den[:, 0:2], p_sb[:, :, lo:256],
                                            mybir.AxisListType.X, ALU.add)
                    nc.vector.tensor_reduce(den[:, 2:4], p_sb[:, :, 256:385],
                                            mybir.AxisListType.X, ALU.add)
                    nc.vector.reciprocal(rden[:], den[:])
                    for h in range(HPC):
                        ctx = pcx.tile([128, 128], F32, name="cx")
                        lo2 = 0 if i > 0 else 128
                        nc.gpsimd.tensor_scalar(
                            out=p_sb[:, h, lo2:256],
                            in0=p_sb[:, h, lo2:256],
                            scalar1=rden[:, h:h + 1], scalar2=0.5,
                            op0=ALU.mult, op1=ALU.mult)
                        nc.vector.tensor_scalar(
                            out=p_sb[:, h, 256:385],
                            in0=p_sb[:, h, 256:385],
                            scalar1=rden[:, 2 + h:3 + h],
                            scalar2=0.5, op0=ALU.mult, op1=ALU.mult)
                        pt = pptp.tile([128, 4, 128], BF16, name="pt")
                        last15 = (i == NT - 1)
                        if i > 0:
                            nc.tensor.matmul(pt[:, 0, :], p_sb[:, h, 0:128],
                                             ish_sb[:, 0:128],
                                             is_transpose=True,
                                             start=True, stop=False)
                        nc.tensor.matmul(pt[:, 1, :], p_sb[:, h, 128:256],
                                         ish_sb[:, 0:128],
                                         is_transpose=True,
                                         start=(i == 0), stop=False)
                        if last15:
                            nc.tensor.matmul(pt[0:1, 3, :],
                                             p_sb[:, h, 384:385],
                                             ish_sb[:, 0:128],
                                             is_transpose=True,
                                             start=False, stop=False)
                        nc.tensor.matmul(pt[:, 2, :], p_sb[:, h, 256:384],
                                         ish_sb[:, 0:128],
                                         is_transpose=True,
                                         start=False, stop=True)
                        ptsb = scrB.tile([128, 4, 128], BF16, tag="pt", bufs=3)
                        c0 = 0 if i > 0 else 1
                        if h == 0:
                            nc.scalar.copy(ptsb[:, c0:3, :], pt[:, c0:3, :])
                        else:
                            nc.vector.tensor_copy(ptsb[:, c0:3, :],
                                                  pt[:, c0:3, :])
                        if last15:
                            nc.vector.tensor_copy(ptsb[0:1, 3, :],
                                                  pt[0:1, 3, :])
                        if i > 0:
                            nc.tensor.matmul(
                                ctx[:], lvn[:, i - 1, h * 128:(h + 1) * 128],
                                ptsb[:, 0, :], start=True, stop=False)
                        nc.tensor.matmul(ctx[:],
                                         lvn[:, i, h * 128:(h + 1) * 128],
                                         ptsb[:, 1, :], start=(i == 0),
                                         stop=False)
                        nc.tensor.matmul(ctx[:], sinkcv[:, h, :],
                                         ptsb[:, 2, :], start=False,
                                         stop=not last15)
                        if last15:
                            nc.tensor.matmul(ctx[:], cv127[:],
                                             ptsb[0:1, 3, :], start=False,
                                             stop=True)
                        if h == 0:
                            nc.scalar.copy(mgT[:, h, i * 128:(i + 1) * 128],
                                           ctx[:])
                        else:
                            nc.vector.tensor_copy(
                                mgT[:, h, i * 128:(i + 1) * 128], ctx[:])

                    # ---- P4 for this s-tile ----
                    o_sb = scrB.tile([128, HID], BF16, tag="o", bufs=3)
                    for qo in range(4):
                        po = pout.tile([128, 512], F32, name="o4")
                        for h in range(HPC):
                            nc.tensor.matmul(
                                po[:], mgT[:, h, i * 128:(i + 1) * 128],
                                wo_sb[:, h, qo * 512:(qo + 1) * 512],
                                start=(h == 0), stop=(h == HPC - 1))
                        # 1024 cols on ACT, 1024 on DVE (engine balance)
                        off = qo * 512
                        if qo < 2:
                            nc.scalar.copy(o_sb[:, off:off + 512], po[:])
                        else:
                            nc.vector.tensor_copy(o_sb[:, off:off + 512],
                                                  po[:])
                    nc.sync.dma_start(out=out_p.ap()[i], in_=o_sb[:])

    nc.compile()
    return nc


def _host_prep(inputs):
    """Build the 8 per-core input maps from full inputs."""
    hs = np.asarray(inputs["hidden_states"], np.float32)[0]  # [S, HID]
    Wq = np.asarray(inputs["Wq"], np.float32)
    Wc = np.asarray(inputs["Wc"], np.float32)
    Wk = np.asarray(inputs["Wk"], np.float32)
    Wv = np.asarray(inputs["Wv"], np.float32)
    Wlk = np.asarray(inputs["Wlk"], np.float32)
    Wlv = np.asarray(inputs["Wlv"], np.float32)
    qn_w = np.asarray(inputs["qn_w"], np.float32)
    kn_w = np.asarray(inputs["kn_w"], np.float32)
    sink_k = np.asarray(inputs["sink_k"], np.float32)
    sink_v = np.asarray(inputs["sink_v"], np.float32)
    Wo = np.asarray(inputs["Wo"], np.float32)
    bq = np.asarray(inputs["bq"], np.float32)
    bc = np.asarray(inputs["bc"], np.float32)
    bk = np.asarray(inputs["bk"], np.float32)
    bv = np.asarray(inputs["bv"], np.float32)
    blk = np.asarray(inputs["blk"], np.float32)
    blv = np.asarray(inputs["blv"], np.float32)

    hT_t = np.ascontiguousarray(hs.T.reshape(KT, 128, S)).astype(NPBF)
    hN_t = hs.reshape(NT, 128, HID).astype(NPBF)

    # host-side compressor softmax (0.016% of FLOPs)
    cw = hs @ Wc[:, 0] + bc[0]                      # [S]
    cwb = cw.reshape(C, R)
    e = np.exp(cwb - cwb.max(axis=1, keepdims=True))
    w = e / e.sum(axis=1, keepdims=True)            # [C, R]
    s_idx = np.arange(S)
    wbig = np.zeros((128, NT, 128), np.float32)
    wbig[s_idx % 128, s_idx // 128, s_idx // R] = w[s_idx // R, s_idx % R]
    wbig = wbig.reshape(128, NT * 128).astype(NPBF)

    # rope tables [freq, pos]
    pos = np.arange(S, dtype=np.float32)
    inv_freq = 1.0 / (10000.0 ** (np.arange(HALF, dtype=np.float32) * 2.0 / ROPE))
    ang = inv_freq[:, None] * pos[None, :]          # [HALF, S]
    tabs = np.concatenate([np.cos(ang), np.sin(ang)], axis=0).astype(NPBF)

    # ck rope tables at block-end positions (kn_w folded)
    pos_c = (np.arange(C, dtype=np.float32) * R + (R - 1))
    angc = pos_c[:, None] * inv_freq[None, :]       # [C, HALF]
    cosc, sinc = np.cos(angc), np.sin(angc)
    kw1, kw2 = kn_w[0:HALF], kn_w[HALF:ROPE]
    ckro = np.concatenate([
        cosc * kw1[None, :], sinc * kw2[None, :],
        sinc * kw1[None, :], cosc * kw2[None, :],
        np.broadcast_to(kn_w[ROPE:][None, :], (C, ROPE)),
    ], axis=1).astype(np.float32)

    # identity | shift(+1)
    r = np.arange(128)
    ident = (r[:, None] == r[None, :]).astype(np.float32)
    shift = (r[None, :] == r[:, None] + 1).astype(np.float32)
    ish = np.concatenate([ident, shift], axis=1).astype(NPBF)

    # masks (multiplicative 0/1)
    p = r[:, None]
    j = np.arange(256)[None, :]
    loc01 = np.where(j < 128, (j >= p), (j - 128 <= p)).astype(np.float32)
    loc01 = loc01.astype(NPBF)
    cm = np.zeros((128, NT, 129), np.float32)
    cm[:, :, 0] = 1.0
    ii = np.arange(NT)[None, :, None]
    cc = np.arange(128)[None, None, :]
    cm[:, :, 1:] = (cc * R + (R - 1) <= ii * 128 + p[:, None]).astype(np.float32)
    cm = cm.reshape(128, NT * 129).astype(NPBF)

    wkv_h = np.ascontiguousarray(
        np.concatenate([Wk, Wv], axis=1).reshape(KT, 128, 256)
        .transpose(1, 0, 2)).reshape(128, KT * 256).astype(NPBF)

    common = dict(hT=hT_t, hN=hN_t, tabs=tabs, wbig=wbig, wkv=wkv_h,
                  bkv=np.concatenate([bk, bv])[None, :].astype(NPBF),
                  ckro=ckro, ish=ish, loc01=loc01, cm01=cm)

    Wq4 = Wq.reshape(HID, NH, HD)
    Wlk4 = Wlk.reshape(HID, NH, HD)
    Wlv4 = Wlv.reshape(HID, NH, HD)
    bq4 = bq.reshape(NH, HD)
    blk4 = blk.reshape(NH, HD)
    blv4 = blv.reshape(NH, HD)
    Wo4 = Wo.reshape(NH, HD, HID)

    def tchunk(wcol):  # [HID, 128] -> [128, 2048] transposed-chunk layout
        return np.ascontiguousarray(
            wcol.reshape(KT, 128, 128).transpose(1, 0, 2)).reshape(128, 2048)

    in_maps = []
    for c in range(NCORES):
        hh = [HPC * c + h for h in range(HPC)]
        w6 = np.stack([
            tchunk(Wq4[:, hh[0]]), tchunk(Wq4[:, hh[1]]),
            tchunk(Wlk4[:, hh[0]]), tchunk(Wlk4[:, hh[1]]),
            tchunk(Wlv4[:, hh[0]]), tchunk(Wlv4[:, hh[1]]),
        ]).astype(NPBF)
        b6w = np.stack([bq4[hh[0]], bq4[hh[1]], blk4[hh[0]], blk4[hh[1]],
                        blv4[hh[0]], blv4[hh[1]], qn_w, kn_w],
                       axis=1).astype(np.float32)
        wo_c = np.ascontiguousarray(
            Wo4[hh].transpose(1, 0, 2)).reshape(128, HPC * HID).astype(NPBF)
        m = dict(common)
        m.update(w6=w6, b6w=b6w, wo=wo_c,
                 skT=sink_k[hh].T.astype(NPBF),
                 sv=sink_v[hh].astype(NPBF))
        in_maps.append(m)
    return in_maps


def kernel(**inputs):
    if "nc" not in _CACHE:
        _CACHE["nc"] = _build_bass()
    nc = _CACHE["nc"]
    in_maps = _host_prep(inputs)
    res = run_bass_kernel_spmd(nc, in_maps, core_ids=list(range(NCORES)))
    out = np.zeros((S, HID), np.float64)
    for c in range(NCORES):
        out += res.results[c]["out_p"].reshape(S, HID).astype(np.float64)
    out += np.asarray(inputs["bo"], np.float32)[None, :]
    return out[None].astype(np.float32)
